# revision 18
# baseline (speedup 1.0000x reference)
"""MinibatchDiscrimination kernel for 8 Trainium2 NeuronCores (v2).

ref:  act = einsum('bf,kfd->bkd', x, kernel)          [256,100,50]
      AD[b,k,j] = sum_d |act[b,k,d] - act[j,k,d]|     [256,100,256]
      f[b,k] = sum_j exp(-AD[b,k,j])                  [256,100]
      out = concat([x, f], 1)                         [256,1124]

v2 design (each unordered pair computed exactly ONCE globally):
  - Triangle pairing: global row i anchors window {i+1..i+W_i} (mod 256,
    W=128 for i<128 else 127).  Core c owns anchors {c, c+8, ..., c+248};
    its xt columns are rotated by c so anchors sit at fixed cols {8m} and
    windows are contiguous in a 384-wide (wrap-duplicated) act layout --
    one SPMD program for all cores.
  - act' = S*act (S=8) via fp8e4 DoubleRow einsum; kd laid out in 40
    128-blocks.  Block 39 partitions 28..127 carry S*w2 (w2[f,k]=sum_d
    kernel), so C'[k,j] = sum_d act' rides the einsum for free.
  - |a-b| = 2max(a,b)-a-b on DVE (fused scalar_tensor_tensor, 16 anchors
    per instr via stride-8 + broadcast APs) and Pool (tensor_scalar max);
    |a-b| = 2relu(a-b)-(a-b) on ScalarE (Relu w/ per-partition bias).
    Tiles are fp8e4; per-anchor selection contraction over kd uses
    DoubleRow fp8 matmuls (2 blocks/instr), accumulating
    P2' - C'_j in PSUM (the -C'_j via a shifted -1 identity matmul over
    act block 39).  AD = psum - C_i + 2CR_i with CR = sum over the
    relu-assigned blocks; the per-anchor term lands in the Exp bias
    (s' = C' - 2CR' via 1 shift + len(R) selC matmuls).
  - exp: ScalarE Exp(scale=-1/S, bias=s'_i/S) straight from PSUM with
    accum_out -> f_own; mirror contributions f_j += et via identity
    matmuls accumulated into a 384-wide PSUM strip (memset once,
    start=False), un-rotated and summed on host.  Diagonal exp(0)=1 is
    analytic (+1 on host), so no exactness tricks are needed anywhere.
"""

import numpy as np
import ml_dtypes
from contextlib import ExitStack

import concourse.bass as bass
import concourse.tile as tile
from concourse import bacc, mybir
from concourse.bass_utils import run_bass_kernel_spmd

B, F, NK, KD = 256, 1024, 100, 50
NCORES = 8
NBLK = 40
FB = 8
S = 8.0
BF16 = mybir.dt.bfloat16
F32 = mybir.dt.float32
FP8 = mybir.dt.float8e4

# Interleaved block->engine assignment (period 8: 5x DVE + 3x fp8) so the
# three tile producers run concurrently against PE's in-order bank
# accumulation.  DR pairs = consecutive FP8_BLOCKS entries (they are
# adjacent in the self8 tensor); the odd 15th fp8 block is a plain fp8
# single matmul.
DVE_BLOCKS = [b for b in range(NBLK) if b % 8 < 5]        # 25, bf16 relu (ts 4x)
FP8_BLOCKS = [b for b in range(NBLK) if b % 8 >= 5]       # 15 fp8 blocks
POOL_BLOCKS = [b for k, b in enumerate(FP8_BLOCKS)
               if (k % 3 != 2) if k < 12] + [FP8_BLOCKS[12]]   # 9, max tiles
ACT_BLOCKS = [b for b in FP8_BLOCKS if b not in POOL_BLOCKS]   # 6, relu tiles
# act psum->sbuf copy engine per block (tunable): ~13 on vector, rest scalar
COPY_ENGINE = {}
for b in range(NBLK):
    COPY_ENGINE[b] = "vector" if b % 3 == 0 else "scalar"


_cached_nc = None


def _emit(ctx, tc, kt, xt, sel, selbf_in, selc, negsh, shift, ident, ft_out,
          mir_out, dbg=None):
    nc = tc.nc
    big = ctx.enter_context(tc.tile_pool(name="big", bufs=1))
    tbbp = ctx.enter_context(tc.tile_pool(name="tbb", bufs=6))
    tb8p = ctx.enter_context(tc.tile_pool(name="tb8", bufs=4))
    etp = ctx.enter_context(tc.tile_pool(name="et", bufs=2))
    pe_ein = ctx.enter_context(tc.tile_pool(name="ps_e", bufs=2, space="PSUM"))
    pe_wave = ctx.enter_context(tc.tile_pool(name="ps_w", bufs=4, space="PSUM"))
    pe_mp = ctx.enter_context(tc.tile_pool(name="ps_m", bufs=1, space="PSUM"))
    pe_sp = ctx.enter_context(tc.tile_pool(name="ps_s", bufs=1, space="PSUM"))

    kt_sb = big.tile([128, NBLK, FB, 128], FP8)
    xt_sb = big.tile([128, FB, 384], FP8)
    self8 = big.tile([128, len(FP8_BLOCKS), 128], FP8)   # sel2, FP8_BLOCKS order
    selbf = big.tile([128, len(DVE_BLOCKS), 128], BF16)  # sel2, DVE_BLOCKS order
    selcm = big.tile([128, len(POOL_BLOCKS), 128], BF16)  # +2 on POOL blocks
    negsh_sb = big.tile([128, 128], BF16)
    shift_sb = big.tile([128, 128], BF16)
    id_sb = big.tile([128, 128], BF16)
    act = big.tile([128, NBLK, 384], BF16)
    anchf = big.tile([128, NBLK, 32], F32)    # +anchor cols (DVE sub / Pool max)
    neganch = big.tile([128, len(ACT_BLOCKS), 32], F32)
    s_sb = big.tile([100, 256], F32)
    ft = big.tile([100, 32], F32)
    mir_sb = big.tile([100, 384], F32)

    # --- input DMA, spread across queues, kt smallest-first ---
    nc.sync.dma_start(xt_sb[:], xt[:])
    nc.scalar.dma_start(self8[:], sel[:])
    nc.scalar.dma_start(selbf[:], selbf_in[:])
    nc.scalar.dma_start(selcm[:], selc[:])
    nc.scalar.dma_start(negsh_sb[:], negsh[:])
    nc.scalar.dma_start(shift_sb[:], shift[:])
    nc.scalar.dma_start(id_sb[:], ident[:])
    kt_chunks = [1, 2, 4, 7, 8, 9, 9]
    off = 0
    for i, ch in enumerate(kt_chunks):
        nc.sync.dma_start(kt_sb[:, off:off + ch], kt[:, off:off + ch])
        off += ch

    act_ps = act[:, 0, 0:1].ap[0][0]    # partition stride (elements)

    # --- phase 1: fp8 DoubleRow einsum + act copies ---
    for b in range(NBLK):
        eins = pe_ein.tile([128, 384], F32, name=f"ein{b % 2}", tag="ein")
        for u in range(4):
            nc.tensor.matmul(
                eins[:], kt_sb[:, b, 2 * u:2 * u + 2, :], xt_sb[:, 2 * u:2 * u + 2, :],
                start=(u == 0), stop=(u == 3),
                perf_mode=mybir.MatmulPerfMode.DoubleRow,
            )
        eng = COPY_ENGINE[b]
        if eng == "scalar":
            nc.scalar.copy(act[:, b, :], eins[:])
        elif eng == "gpsimd":
            nc.gpsimd.tensor_copy(act[:, b, :], eins[:])
        else:
            nc.vector.tensor_copy(act[:, b, :], eins[:])
        if b in ACT_BLOCKS:
            nc.vector.tensor_scalar_mul(
                neganch[:, ACT_BLOCKS.index(b), :], act[:, b, 0:256:8], -1.0)
        else:
            nc.vector.tensor_copy(anchf[:, b, :], act[:, b, 0:256:8])

    # --- s' = C' - 2*CR' at anchor cols; bias = s'/S ---
    # s' = 2*CM' - C'  (M = POOL blocks); bias = s'/S
    sp = pe_sp.tile([100, 256], F32, name="sp", tag="sp")
    nc.tensor.matmul(sp[:], negsh_sb[:, 0:100], act[:, 39, 0:256],
                     start=True, stop=False)
    for i, rb in enumerate(POOL_BLOCKS):
        nc.tensor.matmul(sp[:], selcm[:, i, 0:100], act[:, rb, 0:256],
                         start=False, stop=(i == len(POOL_BLOCKS) - 1))
    nc.vector.tensor_scalar_mul(s_sb[:], sp[:], 1.0 / S)

    # --- mirror psum strip, accumulated across both super-waves ---
    mp = pe_mp.tile([100, 384], F32, name="mp", tag="mp")
    nc.vector.memset(mp[:], 0.0)

    # --- phase 2: two super-waves of 16 anchors (4 bank-groups of 4) ---
    # Each 4-anchor group owns one full 2KB PSUM bank [100, 4, 128] so its
    # accumulation group never shares a PSUM zero-region with another group.
    # Tiles: DVE blocks -> bf16 relu via plain tensor_scalar (4x mode),
    # consumed by per-block bf16 matmuls; Pool (fp8 max) / Act (fp8 relu)
    # blocks consumed by DoubleRow fp8 pair matmuls (block 25 as a plain
    # fp8 single).  SW1 computes a throwaway 128th column per anchor (pair
    # (i, i+128) is owned by anchor i-128) so APs stay simple; exp/mirror
    # read only the first 127 columns there.
    for sw in range(2):
        Weff = 128 if sw == 0 else 127
        c0 = 128 * sw                       # first anchor col of this sw
        et = etp.tile([100, 16, 128], BF16, name="et", tag="et")
        if sw == 0 and dbg is not None:
            et_dbg = et
        waveps = [
            pe_wave.tile([100, 4, 128], F32, name=f"wv{sw}_{g}", tag="wv")
            for g in range(4)
        ]
        tb8_cur = None
        for b in range(NBLK):
            if b in DVE_BLOCKS:
                di = DVE_BLOCKS.index(b)
                tbb = tbbp.tile([128, 16, 128], BF16, name="tbb", tag="tbb")
                for m16 in range(16):
                    a0 = c0 + 8 * m16
                    win = bass.AP(act[:, b, 0:1].tensor,
                                  act[:, b, 0:1].offset + a0 + 1,
                                  [[act_ps, 128], [1, 128]])
                    nc.vector.tensor_scalar(
                        tbb[:, m16, :], win,
                        anchf[:, b, 16 * sw + m16:16 * sw + m16 + 1], 0.0,
                        mybir.AluOpType.subtract, mybir.AluOpType.max,
                    )
                for g in range(4):
                    nc.tensor.matmul(
                        waveps[g][:], selbf[:, di, 0:100],
                        tbb[:, 4 * g:4 * g + 4, :],
                        start=(b == 0), stop=False,
                    )
            else:
                q = FP8_BLOCKS.index(b)
                h = q % 2
                last_single = (q == len(FP8_BLOCKS) - 1 and len(FP8_BLOCKS) % 2 == 1)
                if h == 0:
                    tb8 = tb8p.tile([128, 2, 16, 128], FP8, name="tb8", tag="tb8")
                    tb8_cur = tb8
                else:
                    tb8 = tb8_cur
                if b in ACT_BLOCKS:
                    abi = ACT_BLOCKS.index(b)
                    for m16 in range(16):
                        a0 = c0 + 8 * m16
                        nc.scalar.activation(
                            tb8[:, h, m16, :], act[:, b, a0 + 1:a0 + 129],
                            mybir.ActivationFunctionType.Relu,
                            bias=neganch[:, abi, 16 * sw + m16:16 * sw + m16 + 1],
                            scale=1.0,
                        )
                else:
                    for m16 in range(16):
                        a0 = c0 + 8 * m16
                        nc.gpsimd.tensor_scalar(
                            tb8[:, h, m16, :], act[:, b, a0 + 1:a0 + 129],
                            anchf[:, b, 16 * sw + m16:16 * sw + m16 + 1], 0.0,
                            mybir.AluOpType.max, mybir.AluOpType.add,
                        )
                if last_single:
                    for g in range(4):
                        nc.tensor.matmul(
                            waveps[g][:], self8[:, q, 0:100],
                            tb8[:, 0, 4 * g:4 * g + 4, :],
                            start=False, stop=False, skip_group_check=True,
                        )
                elif h == 1:
                    for g in range(4):
                        nc.tensor.matmul(
                            waveps[g][:], self8[:, q - 1:q + 1, 0:100],
                            tb8[:, :, 4 * g:4 * g + 4, :],
                            start=False, stop=False,
                            perf_mode=mybir.MatmulPerfMode.DoubleRow,
                            skip_group_check=True,
                        )
        for g in range(4):
            a0g = c0 + 32 * g
            abase39 = act[:, 39, 0:1]
            fold_rhs = bass.AP(abase39.tensor, abase39.offset + a0g + 1,
                               [[act_ps, 128], [8, 4], [1, 128]])
            nc.tensor.matmul(
                waveps[g][:], negsh_sb[:, 0:100], fold_rhs,
                start=False, stop=True, skip_group_check=True,
            )
            for gi in range(4):
                m16 = 4 * g + gi
                m = 16 * sw + m16
                a0 = c0 + 8 * m16
                nc.scalar.activation(
                    et[:, m16, 0:Weff], waveps[g][:, gi, 0:Weff],
                    mybir.ActivationFunctionType.Exp,
                    bias=s_sb[:, a0:a0 + 1], scale=-1.0 / S,
                    accum_out=ft[:, m:m + 1],
                )
                nc.tensor.matmul(
                    mp[:, a0 + 1:a0 + 1 + Weff], id_sb[0:100, 0:100],
                    et[:, m16, 0:Weff],
                    start=False, stop=(sw == 1 and m16 == 15),
                    skip_group_check=True,
                )

    nc.vector.tensor_copy(mir_sb[:], mp[:])
    nc.sync.dma_start(ft_out[:], ft[:])
    nc.sync.dma_start(mir_out[:], mir_sb[:])
    if dbg is not None:
        actdump, sdump, etdump = dbg
        nc.sync.dma_start(actdump[:], act[:])
        nc.sync.dma_start(sdump[:], s_sb[:])
        nc.sync.dma_start(etdump[:], et_dbg[:])


def _build():
    global _cached_nc
    if _cached_nc is None:
        nc = bacc.Bacc(
            "TRN2",
            target_bir_lowering=False,
            debug=False,
            enable_asserts=False,
            num_devices=NCORES,
        )
        kt_d = nc.dram_tensor("kt", [128, NBLK, FB, 128], FP8, kind="ExternalInput")
        xt_d = nc.dram_tensor("xt", [128, FB, 384], FP8, kind="ExternalInput")
        sel_d = nc.dram_tensor("sel", [128, len(FP8_BLOCKS), 128], FP8, kind="ExternalInput")
        selbf_d = nc.dram_tensor("selbf", [128, len(DVE_BLOCKS), 128], BF16, kind="ExternalInput")
        selc_d = nc.dram_tensor("selc", [128, len(POOL_BLOCKS), 128], BF16, kind="ExternalInput")
        negsh_d = nc.dram_tensor("negsh", [128, 128], BF16, kind="ExternalInput")
        shift_d = nc.dram_tensor("shift", [128, 128], BF16, kind="ExternalInput")
        id_d = nc.dram_tensor("ident", [128, 128], BF16, kind="ExternalInput")
        ft_d = nc.dram_tensor("ft", [100, 32], F32, kind="ExternalOutput")
        mir_d = nc.dram_tensor("mir", [100, 384], F32, kind="ExternalOutput")
        with tile.TileContext(nc) as tc, ExitStack() as ctx:
            _emit(ctx, tc, kt_d.ap(), xt_d.ap(), sel_d.ap(), selbf_d.ap(),
                  selc_d.ap(), negsh_d.ap(), shift_d.ap(), id_d.ap(),
                  ft_d.ap(), mir_d.ap())
        nc.compile()
        _cached_nc = nc
    return _cached_nc


def _prep_shared(w):
    kT = w.transpose(1, 0, 2).reshape(F, NK * KD)
    ktfull = np.zeros((F, NBLK * 128), np.float32)
    ktfull[:, :NK * KD] = S * kT
    ktfull[:, 39 * 128 + 28:39 * 128 + 128] = S * w.sum(axis=2).T
    kt_host = np.ascontiguousarray(
        ktfull.reshape(FB, 128, NBLK, 128).transpose(1, 2, 0, 3)
    ).astype(ml_dtypes.float8_e4m3fn)

    kd_ids = np.arange(NK * KD)
    sel2 = np.zeros((NBLK * 128, 128), np.float32)
    sel2[kd_ids, kd_ids // KD] = 2.0
    sel2_pbk = sel2.reshape(NBLK, 128, 128).transpose(1, 0, 2)  # [p, b, k]
    sel_host = np.ascontiguousarray(
        sel2_pbk[:, FP8_BLOCKS, :]).astype(ml_dtypes.float8_e4m3fn)
    selbf_host = np.ascontiguousarray(
        sel2_pbk[:, DVE_BLOCKS, :]).astype(ml_dtypes.bfloat16)

    selc = np.zeros((NBLK * 128, 128), np.float32)
    for bidx in POOL_BLOCKS:
        lo, hi = bidx * 128, min(bidx * 128 + 128, NK * KD)
        ids = np.arange(lo, hi)
        selc[ids, ids // KD] = 2.0
    selc_host = np.ascontiguousarray(
        selc.reshape(NBLK, 128, 128).transpose(1, 0, 2)[:, POOL_BLOCKS, :]
    ).astype(ml_dtypes.bfloat16)

    negsh = np.zeros((128, 128), np.float32)
    negsh[28 + np.arange(NK), np.arange(NK)] = -1.0
    shift = np.zeros((128, 128), np.float32)
    shift[28 + np.arange(NK), np.arange(NK)] = 1.0
    ident = np.zeros((128, 128), np.float32)
    ident[np.arange(NK), np.arange(NK)] = 1.0
    return (kt_host, sel_host, selbf_host, selc_host,
            negsh.astype(ml_dtypes.bfloat16), shift.astype(ml_dtypes.bfloat16),
            ident.astype(ml_dtypes.bfloat16))


def kernel(x, kernel, _trace=False):
    x = np.asarray(x, dtype=np.float32)
    w = np.asarray(kernel, dtype=np.float32)
    nc = _build()
    (kt_host, sel_host, selbf_host, selc_host, negsh_host, shift_host,
     id_host) = _prep_shared(w)
    in_maps = []
    for c in range(NCORES):
        rot = (np.arange(256) + c) % 256
        x384 = x[np.concatenate([rot, rot[:128]])]          # [384, F]
        xt_host = np.ascontiguousarray(
            x384.T.reshape(FB, 128, 384).transpose(1, 0, 2)
        ).astype(ml_dtypes.float8_e4m3fn)
        in_maps.append({
            "kt": kt_host, "xt": xt_host, "sel": sel_host, "selbf": selbf_host,
            "selc": selc_host, "negsh": negsh_host, "shift": shift_host,
            "ident": id_host,
        })
    res = run_bass_kernel_spmd(
        nc, in_maps, core_ids=list(range(NCORES)), trace=_trace
    )
    f_acc = np.zeros((NK, B), np.float64)
    for c in range(NCORES):
        rot = (np.arange(256) + c) % 256
        mir = np.asarray(res.results[c]["mir"], np.float64)
        mir256 = mir[:, :256].copy()
        mir256[:, :128] += mir[:, 256:384]
        f_acc[:, rot] += mir256
        f_acc[:, (np.arange(32) * 8 + c) % 256] += np.asarray(
            res.results[c]["ft"], np.float64)
    f_full = (f_acc.T + 1.0).astype(np.float32)
    out = np.concatenate([x, f_full], axis=1)
    if _trace:
        return out, res
    return out
